# revision 1
# baseline (speedup 1.0000x reference)
"""Causal self-attention, 8 TRN2 cores, head-parallel + pipelined I/O.

Host I/O is minimized and pipelined: each core uploads 256 tokens of each
batch (core r, batch b: tokens [b*2048 + r*256, b*2048 + (r+1)*256)); two
on-device AllGathers (one per batch) rebuild the full activation so batch-0
compute overlaps batch-1's gather. The output projection is row-split
Megatron-style with bp/8 folded into every partial; four half-batch
ReduceScatters emit finished bf16 rows that upload while later chunks
compute. All ExternalInput pulls are DRAM-contiguous blocks split across
the SP and ACT DMA queues; output DMAs issue from the Pool queue, which is
idle between collectives.

Per-core inputs (all pulls contiguous):
  xtl    [B*128, CO*TPB] bf16  row b*128+p, col co*256+t' = x.T[co*128+p, tok]
  wqkvt  [CO*128, 384]   bf16  [Wq_i.T | Wk_i.T | Wv_i.T], row co*128+p
  bqkv   [1, 384]        f32   (q,k,v) per channel, one burst
  wpt    [128, 1024]     bf16  Wp[:, ch_i].T
  bp8    [1, 1024]       f32   bp / 8
Output per core:
  out    [512, 1024]     bf16  row (b*2+h)*128+i = global b*2048+h*1024+cid*128+i
"""

import sys

sys.path.insert(0, "/opt/trn_rl_repo")

import numpy as np
import ml_dtypes

import concourse.bass as bass
import concourse.mybir as mybir
import concourse.tile as tile
from concourse import bacc
from concourse.bass_utils import run_bass_kernel_spmd
from concourse.masks import make_identity

bf16 = ml_dtypes.bfloat16
B, T, C, H = 2, 2048, 1024, 16
HD = C // H              # 64
NCORE = 8
BT = B * T               # 4096
TPB = T // NCORE         # 256 tokens per core per batch (I/O shard)
HPC = H // NCORE         # 2 heads per core
CW = HPC * HD            # 128 channels per core
QB = 512                 # q-block width
NQB = T // QB            # 4 q-blocks per batch
KP = 128                 # k-panel width
SCALE = 1.0 / 8.0        # 1/sqrt(64)
CO = C // 128            # 8 contraction chunks

f32 = mybir.dt.float32
bf = mybir.dt.bfloat16
AF = mybir.ActivationFunctionType
ALU = mybir.AluOpType
GROUP = [list(range(NCORE))]

_cached_nc = None


def _build(reps=1, phase="full"):
    do_qkv = phase in ("qkv", "attn", "full")
    do_attn = phase in ("attn", "full")
    do_proj = phase == "full"
    shared = "Shared" if reps == 1 else "Local"

    nc = bacc.Bacc("TRN2", target_bir_lowering=False, debug=False, num_devices=NCORE)
    xtl_d = nc.dram_tensor("xtl", [B * 128, CO * TPB], bf, kind="ExternalInput")
    wqkvt_d = nc.dram_tensor("wqkvt", [CO * 128, 3 * CW], bf, kind="ExternalInput")
    bqkv_d = nc.dram_tensor("bqkv", [1, 3 * CW], f32, kind="ExternalInput")
    wpt_d = nc.dram_tensor("wpt", [CW, C], bf, kind="ExternalInput")
    bp8_d = nc.dram_tensor("bp8", [1, C], f32, kind="ExternalInput")
    out_d = nc.dram_tensor("out", [B * TPB, C], bf, kind="ExternalOutput")

    with tile.TileContext(nc) as tc:
        with tc.tile_pool(name="const", bufs=1) as cp, \
             tc.tile_pool(name="dram", bufs=1, space="DRAM") as dp, \
             tc.tile_pool(name="work", bufs=5) as wp, \
             tc.tile_pool(name="mm", bufs=2, space="PSUM") as mmp, \
             tc.tile_pool(name="stp", bufs=3, space="PSUM") as stp, \
             tc.tile_pool(name="otp", bufs=2, space="PSUM") as otp, \
             tc.tile_pool(name="bcp", bufs=1, space="PSUM") as bcp:

            # ---- persistent tiles ----
            xt = cp.tile([128, CO, BT], bf)            # x.T, channel chunks
            wqkv = cp.tile([128, CO, 3 * CW], bf)
            bqkv = cp.tile([CW, 3], f32)
            wpt = cp.tile([CW, C], bf)
            bias8 = cp.tile([128, C], f32)             # broadcast bp/8
            bp8 = cp.tile([1, C], f32)
            qT = cp.tile([128, BT], bf)
            kT = cp.tile([128, BT], bf)
            vT = cp.tile([128, BT], bf)
            vnat = cp.tile([128, B * HPC, T // KP, HD + 1], bf)
            yT = cp.tile([128, BT], bf)
            ident = cp.tile([128, 128], bf)
            ones65 = cp.tile([HD + 1, HD], bf)
            onesr = cp.tile([1, 128], f32)
            # causal masks for the 4 diagonal-panel offsets, as column
            # slices of one [128, 896] tile: mask[p, u] = (u >= p + 384).
            # Diagonal panel joff uses cols [384-128*joff, 896-128*joff).
            NDIAG = QB // KP
            maskw = QB + (NDIAG - 1) * KP
            maskbig = cp.tile([128, maskw], bf)

            # DRAM bounce buffers for collectives
            xin0 = dp.tile([128, CO * TPB], bf)
            xin1 = dp.tile([128, CO * TPB], bf)
            xins = (xin0, xin1)
            xg0 = dp.tile([NCORE * 128, CO * TPB], bf, addr_space=shared)
            xg1 = dp.tile([NCORE * 128, CO * TPB], bf, addr_space=shared)
            bq_bounce = dp.tile([1, 3 * CW], f32)
            pout_b = dp.tile([BT, C], bf)
            rs_b = dp.tile([B * TPB, C], bf)
            xgs = (xg0, xg1)

            if not do_qkv:
                nc.gpsimd.memset(qT[:], 0.0)
                nc.gpsimd.memset(kT[:], 0.0)
                nc.gpsimd.memset(vT[:], 0.0)
            if not do_attn:
                nc.gpsimd.memset(yT[:], 0.0)
                nc.gpsimd.memset(vnat[:], 0.0)

            for _rep in range(reps):
                # ---- x: PCIe pull of own slices (split across the SP
                # and ACT DMA queues to halve the latency before the first
                # AllGather), then per-batch AllGather ----
                for b in range(B):
                    src_b = xtl_d.ap()[b * 128:(b + 1) * 128, :]
                    nc.sync.dma_start(xins[b][:64, :], src_b[:64, :])
                    nc.scalar.dma_start(xins[b][64:, :], src_b[64:, :])
                    nc.gpsimd.collective_compute(
                        "AllGather", ALU.bypass, replica_groups=GROUP,
                        ins=[xins[b][:].opt()],
                        outs=[xgs[b][:].opt()],
                    )

                # const setup AFTER the gather launches so the Pool queue
                # fires AllGather_0 as early as possible
                make_identity(nc, ident[:])
                nc.gpsimd.memset(ones65[:], 1.0)
                nc.gpsimd.memset(onesr[:], 1.0)
                nc.gpsimd.memset(maskbig[:], 1.0)
                nc.gpsimd.affine_select(
                    out=maskbig[:], in_=maskbig[:],
                    compare_op=ALU.is_ge, fill=0.0,
                    base=-(NDIAG - 1) * KP, channel_multiplier=-1,
                    pattern=[[1, maskw]],
                )

                # ---- weights / biases (overlap the gathers; per-chunk
                # pulls so QKV accumulation can start on early chunks) ----
                # bqkv: one contiguous PCIe burst into a DRAM bounce,
                # then an on-chip scatter to [128 partitions, 3]
                nc.sync.dma_start(bq_bounce[:], bqkv_d.ap())
                nc.sync.dma_start(
                    bqkv[:],
                    bq_bounce[:].rearrange("o (p j) -> (o p) j", p=CW))
                nc.sync.dma_start(bp8[:], bp8_d.ap())
                for co in range(CO):
                    eng = nc.sync if co % 2 == 0 else nc.scalar
                    eng.dma_start(
                        wqkv[:, co, :],
                        wqkvt_d.ap()[co * 128:(co + 1) * 128, :])
                nc.scalar.dma_start(wpt[:], wpt_d.ap())
                for half in range(C // QB):
                    ps = bcp.tile([128, QB], f32, tag="bc")
                    nc.tensor.matmul(
                        ps[:], onesr[:], bp8[:, half * QB:(half + 1) * QB],
                        start=True, stop=True,
                    )
                    nc.vector.tensor_copy(
                        bias8[:, half * QB:(half + 1) * QB], ps[:])

                if do_attn:
                    nc.gpsimd.memset(vnat[:, :, :, HD:HD + 1], 1.0)

                for b in range(B):
                  # Scheduling hint: keep batch-1 work behind batch-0
                  # attention in the engine queues — its AllGather finishes
                  # late, and hoisted batch-1 QKV would stall the PE queue.
                  with tc.tile_wait_until(0.25, enable=(b == 1 and reps == 1)):
                    # ---- xg -> SBUF (token-major per co) ----
                    for r in range(NCORE):
                        src = xgs[b][r * 128:(r + 1) * 128, :]
                        src = src.rearrange("p (co n) -> p co n", co=CO)
                        off = b * T + r * TPB
                        nc.sync.dma_start(xt[:, :, off:off + TPB], src)

                    # ---- QKV projections for this batch ----
                    if do_qkv:
                        dsts = (qT, kT, vT)
                        for p in range(3):
                            for n in range(NQB):
                                tok = b * T + n * QB
                                ps = mmp.tile([128, QB], f32, tag="mm")
                                for co in range(CO):
                                    nc.tensor.matmul(
                                        ps[:],
                                        wqkv[:, co, p * CW:(p + 1) * CW],
                                        xt[:, co, tok:tok + QB],
                                        start=(co == 0),
                                        stop=(co == CO - 1),
                                    )
                                nc.vector.tensor_add(
                                    dsts[p][:, tok:tok + QB],
                                    ps[:],
                                    bqkv[:, p:p + 1].to_broadcast((128, QB)),
                                )

                    # ---- V natural layout for this batch ----
                    if do_attn:
                        for h in range(HPC):
                            for kc in range(T // KP):
                                tp = mmp.tile([128, HD], bf, tag="mm")
                                nc.tensor.transpose(
                                    tp[:],
                                    vT[HD * h:HD * (h + 1),
                                       b * T + kc * KP: b * T + (kc + 1) * KP],
                                    ident[HD * h:HD * (h + 1), HD * h:HD * (h + 1)],
                                )
                                nc.vector.tensor_copy(
                                    vnat[:, b * HPC + h, kc, 0:HD], tp[:])

                        # ---- attention for this batch ----
                        for qb in range(NQB):
                            n_kp = (qb + 1) * (QB // KP)
                            q_sl = slice(b * T + qb * QB, b * T + (qb + 1) * QB)
                            ots = []
                            for h in range(HPC):
                                ots.append(otp.tile([HD + 1, QB], f32, tag="ot",
                                                    name=f"ot_{b}_{qb}_{h}"))
                            for j in range(n_kp):
                                k_sl = slice(b * T + j * KP, b * T + (j + 1) * KP)
                                for h in range(HPC):
                                    hsl = slice(HD * h, HD * (h + 1))
                                    st = stp.tile([128, QB], f32, tag="st")
                                    nc.tensor.matmul(
                                        st[:], kT[hsl, k_sl], qT[hsl, q_sl],
                                        start=True, stop=True,
                                    )
                                    pt = wp.tile([128, QB], bf, tag="pt")
                                    nc.scalar.activation(pt[:], st[:], AF.Exp,
                                                         scale=SCALE)
                                    joff = j - qb * (QB // KP)
                                    if joff >= 0:
                                        # causal zeroing on DVE (Pool is
                                        # busy with the collectives)
                                        moff = (NDIAG - 1 - joff) * KP
                                        nc.vector.tensor_mul(
                                            pt[:], pt[:],
                                            maskbig[:, moff:moff + QB])
                                    nc.tensor.matmul(
                                        ots[h][:],
                                        vnat[:, b * HPC + h, j, :],
                                        pt[:],
                                        start=(j == 0),
                                        stop=(j == n_kp - 1),
                                    )
                            # normalize by softmax denominators (last PV row)
                            for h in range(HPC):
                                rec = wp.tile([HD + 1, QB], bf,
                                              tag="rec", name=f"rec_{b}_{qb}_{h}")
                                with nc.allow_low_precision(
                                        reason="bf16 denominator broadcast"):
                                    nc.vector.reciprocal(
                                        rec[HD:HD + 1, :], ots[h][HD:HD + 1, :])
                                bc = bcp.tile([HD, QB], f32, tag="bc",
                                              name=f"bc_{b}_{qb}_{h}")
                                nc.tensor.matmul(
                                    bc[:],
                                    ones65[HD:HD + 1, :],
                                    rec[HD:HD + 1, :],
                                    start=True, stop=True,
                                )
                                ocp = wp.tile([HD, QB], f32, tag="ocp",
                                              name=f"ocp_{b}_{qb}_{h}")
                                nc.vector.tensor_copy(ocp[:], ots[h][0:HD, :])
                                if h == 0:
                                    nc.vector.tensor_mul(
                                        yT[0:HD, q_sl], ocp[:], bc[:])
                                else:
                                    t64 = wp.tile([HD, QB], bf, tag="t64")
                                    nc.vector.tensor_mul(t64[:], ocp[:], bc[:])
                                    nc.sync.dma_start(yT[HD:2 * HD, q_sl], t64[:])

                    # ---- partial projection (+ bp/8) for this batch ----
                    if do_proj:
                        for r in range(T // 128):
                            row0 = b * T + r * 128
                            osb = wp.tile([128, C], bf, tag="osb")
                            for half in range(C // QB):
                                ps = mmp.tile([128, QB], f32, tag="mm")
                                nc.tensor.matmul(
                                    ps[:],
                                    yT[:, row0:row0 + 128],
                                    wpt[:, half * QB:(half + 1) * QB],
                                    start=True, stop=True,
                                )
                                with nc.allow_low_precision(
                                        reason="bf16 output partials"):
                                    nc.vector.tensor_add(
                                        osb[:, half * QB:(half + 1) * QB],
                                        ps[:],
                                        bias8[:, half * QB:(half + 1) * QB])
                            nc.sync.dma_start(
                                pout_b[row0:row0 + 128, :], osb[:])

                    # ---- half-batch ReduceScatter -> own 128 rows -> out ----
                    # chunk (b, h): pout rows [b*T + h*1024, +1024); core c
                    # receives rows b*T + h*1024 + c*128 + [0, 128).
                    for h in range(2):
                        hrow = b * T + h * (T // 2)
                        orow = (b * 2 + h) * 128
                        nc.gpsimd.collective_compute(
                            "ReduceScatter", ALU.add, replica_groups=GROUP,
                            ins=[pout_b[hrow:hrow + T // 2, :].opt()],
                            outs=[rs_b[orow:orow + 128, :].opt()],
                        )
                        # out DMA from the Pool queue (idle between
                        # collectives); keeps SP clear for batch-1's xg loads
                        nc.gpsimd.dma_start(
                            out_d.ap()[orow:orow + 128, :],
                            rs_b[orow:orow + 128, :])

    nc.finalize()
    return nc


def _prep_in_maps(x, Wq, bq, Wk, bk, Wv, bv, Wp, bp):
    # cast first (contiguous, halves the strided-copy traffic), then one
    # batched transpose building every core's upload block at once:
    # xtl_all[r, b, p, co, t'] = x[b*T + r*TPB + t', co*128 + p]
    x2 = x.reshape(B, NCORE, TPB, CO, 128).astype(bf16)
    xtl_all = np.ascontiguousarray(x2.transpose(1, 0, 4, 3, 2))
    xtl_all = xtl_all.reshape(NCORE, B * 128, CO * TPB)
    bp8 = (bp.astype(np.float32) / NCORE).reshape(1, C)
    in_maps = []
    for i in range(NCORE):
        ch = slice(CW * i, CW * (i + 1))
        xtl = xtl_all[i]
        wqkvt = np.ascontiguousarray(np.concatenate(
            [Wq[ch].T, Wk[ch].T, Wv[ch].T], axis=1).astype(bf16))  # [C, 384]
        bqkv = np.stack([bq[ch], bk[ch], bv[ch]],
                        axis=1).astype(np.float32).reshape(1, 3 * CW)
        wpt = np.ascontiguousarray(Wp[:, ch].T).astype(bf16)
        in_maps.append({
            "xtl": xtl,
            "wqkvt": wqkvt,
            "bqkv": np.ascontiguousarray(bqkv),
            "wpt": wpt,
            "bp8": bp8,
        })
    return in_maps


def _assemble(results):
    # results[c]["out"]: [B*TPB, C] bf16, row (b*2 + h)*128 + i holds
    # global row b*2048 + h*1024 + c*128 + i.
    out = np.empty((B, 2, NCORE, 128, C), np.float32)
    for c, r in enumerate(results):
        out[:, :, c] = np.asarray(r["out"]).reshape(B, 2, 128, C)
    return out.reshape(B, T, C)


def kernel(x, Wq, bq, Wk, bk, Wv, bv, Wp, bp):
    global _cached_nc
    x = np.asarray(x, np.float32)
    Wq, bq = np.asarray(Wq, np.float32), np.asarray(bq, np.float32)
    Wk, bk = np.asarray(Wk, np.float32), np.asarray(bk, np.float32)
    Wv, bv = np.asarray(Wv, np.float32), np.asarray(bv, np.float32)
    Wp, bp = np.asarray(Wp, np.float32), np.asarray(bp, np.float32)

    if _cached_nc is None:
        _cached_nc = _build()
    nc = _cached_nc

    in_maps = _prep_in_maps(x, Wq, bq, Wk, bk, Wv, bv, Wp, bp)
    res = run_bass_kernel_spmd(nc, in_maps, core_ids=list(range(NCORE)))
    return _assemble(res.results)

